# revision 25
# baseline (speedup 1.0000x reference)
"""BayesianBatchNorm on 8 TRN2 NeuronCores.

Two sharding strategies:

"channel" (default): each core owns 32 channels across the FULL batch, so
  the batch statistics complete locally — no cross-core communication.
  Host lays the core's data out as 16 tiles of [128, 3136] where
  partition p = (batch_quarter, channel) = (p//32, p%32); a single
  TensorE matmul against a 0/1 selection matrix reduces the per-quarter
  partial sums across partitions AND broadcasts the result back to all
  128 partitions. The KL term for the core's 32 channels ships out and
  the host sums the 8x32 terms into the div scalar.

"batch": data-parallel over batch; per-channel (sum, sumsq) partials are
  AllReduced across cores (2KB payload). Kept for comparison — the
  collective costs 25-55us wall on this environment.

Pass 2 normalizes y = x*scale + shift per channel and streams out.
"""

import sys

sys.path.insert(0, "/opt/trn_rl_repo")

import numpy as np

from concourse import bass, mybir
import concourse.bacc as bacc
import concourse.tile as tile
from concourse import bass_utils
from concourse.bass_interp import get_hw_module

N_CORES = 8
N, C, H, W = 64, 256, 56, 56
FREE = H * W                     # 3136
NT = 16                          # tiles of [128, FREE] per core
CPC = C // N_CORES               # 32 channels per core (channel mode)
NQ = 128 // CPC                  # 4 batch quarters on partitions (channel mode)
NB = N // N_CORES                # 8 batches per core (batch mode)
N_GLOBAL = N * FREE              # 200704 elements per channel
MOMENTUM = 0.1
EPS = 1e-5
JITTER = 1e-5

# consts tensor column layout
(C_A1, C_A2, C_B1, C_B2, C_LRM, C_VS, C_IVS, C_GAM, C_BET, C_Q, C_R) = range(11)
NCONST = 11

# shard: "channel" (no collective) or "batch" (AllReduce).
# in_dt: dtype x is stored in DRAM as (host casts).
# cache: keep x tiles resident in SBUF between the two passes (skips the
# second HBM read of x).
CONFIG = dict(shard="channel", in_dt="float16", cache=True, out_dt="float32")

_ALU = mybir.AluOpType
_AF = mybir.ActivationFunctionType
_F32 = mybir.dt.float32


def _build(shard: str, in_dt: str, cache: bool, variant: str = "full",
           out_dt: str = "float32"):
    ng = 2 if shard == "batch" else 1
    nc = bacc.Bacc("TRN2", debug=False, enable_asserts=False, num_devices=N_CORES)
    xdt = mybir.dt.float16 if in_dt == "float16" else _F32
    ydt = mybir.dt.float16 if out_dt == "float16" else _F32

    x = nc.dram_tensor("x", [NT, 128, FREE], xdt, kind="ExternalInput").ap()
    cvec = nc.dram_tensor("cvec", [128, ng, NCONST], _F32, kind="ExternalInput").ap()
    y = nc.dram_tensor("y", [NT, 128, FREE], ydt, kind="ExternalOutput").ap()
    klp = CPC if shard == "channel" else 128
    kl = nc.dram_tensor("kl", [klp, ng], _F32, kind="ExternalOutput").ap()
    if shard == "channel":
        sel = nc.dram_tensor("sel", [128, 128], _F32, kind="ExternalInput").ap()

    vec = nc.vector
    act = nc.scalar

    with tile.TileContext(nc) as tc:
        with (
            tc.tile_pool(name="xin", bufs=NT if cache else 6) as xpool,
            tc.tile_pool(name="yout", bufs=4) as ypool,
            tc.tile_pool(name="small", bufs=1) as sp,
            tc.tile_pool(name="psum", bufs=1, space="PSUM") as pp,
            tc.tile_pool(name="dram", bufs=1, space="DRAM") as dp,
        ):
            cv = sp.tile([128, ng, NCONST], _F32)
            nc.sync.dma_start(cv[:], cvec)
            if shard == "channel":
                sel_t = sp.tile([128, 128], _F32)
                nc.sync.dma_start(sel_t[:], sel)

            def cc(k):  # [128, ng] column view of the consts
                return cv[:, :, k]

            # ---- pass 1: load tiles, per-tile (sum, sumsq) partials ----
            npart = NT // ng
            sum_part = [
                sp.tile([128, npart], _F32, tag=f"sp{g}", name=f"sum_part{g}")
                for g in range(ng)
            ]
            sq_part = [
                sp.tile([128, npart], _F32, tag=f"qp{g}", name=f"sq_part{g}")
                for g in range(ng)
            ]
            xtiles = []
            for t in range(NT):
                g, nn = t % ng, t // ng
                xt = xpool.tile([128, FREE], xdt, tag="xt")
                (nc.sync if t % 2 == 0 else nc.gpsimd).dma_start(xt[:], x[t])
                vec.reduce_sum(
                    sum_part[g][:, nn : nn + 1], xt[:], axis=mybir.AxisListType.X
                )
                scr = sp.tile(
                    [128, FREE], mybir.dt.float16, tag="scr", bufs=2, name="scr"
                )
                act.activation(
                    scr[:],
                    xt[:],
                    _AF.Square,
                    bias=0.0,
                    scale=1.0,
                    accum_out=sq_part[g][:, nn : nn + 1],
                )
                xtiles.append(xt)

            # ---- combine partials into global per-channel (sum, sumsq) ----
            cc_out = sp.tile([128, 2 * ng], _F32)
            if shard == "channel":
                packed = sp.tile([128, 2], _F32)
                vec.reduce_sum(packed[:, 0:1], sum_part[0][:], axis=mybir.AxisListType.X)
                vec.reduce_sum(packed[:, 1:2], sq_part[0][:], axis=mybir.AxisListType.X)
                # one matmul: sel[k,m]=1 iff k%32==m%32 reduces the 4 batch
                # quarters AND broadcasts back to all 128 partitions
                ps = pp.tile([128, 2], _F32)
                nc.tensor.matmul(ps[:], sel_t[:], packed[:], start=True, stop=True)
                vec.tensor_copy(cc_out[:], ps[:])
            else:
                cc_in = sp.tile([128, 2 * ng], _F32)
                for g in range(ng):
                    vec.reduce_sum(
                        cc_in[:, g : g + 1], sum_part[g][:], axis=mybir.AxisListType.X
                    )
                    vec.reduce_sum(
                        cc_in[:, ng + g : ng + g + 1],
                        sq_part[g][:],
                        axis=mybir.AxisListType.X,
                    )
                bounce_in = dp.tile([128, 2 * ng], _F32)
                bounce_out = dp.tile([128, 2 * ng], _F32)
                nc.gpsimd.dma_start(bounce_in[:], cc_in[:])
                if "nocc" in variant:
                    nc.gpsimd.dma_start(bounce_out[:], bounce_in[:])
                else:
                    nc.gpsimd.collective_compute(
                        "AllReduce",
                        _ALU.add,
                        replica_groups=[list(range(N_CORES))],
                        ins=[bounce_in.opt()],
                        outs=[bounce_out.opt()],
                    )
                nc.gpsimd.dma_start(cc_out[:], bounce_out[:])

            # ---- finalize: all [128, ng] elementwise ----
            sums, sqs = cc_out[:, 0:ng], cc_out[:, ng : 2 * ng]
            mean = sp.tile([128, ng], _F32)
            e2 = sp.tile([128, ng], _F32)
            bvar = sp.tile([128, ng], _F32)
            rmt = sp.tile([128, ng], _F32)
            rvt = sp.tile([128, ng], _F32)
            d = sp.tile([128, ng], _F32)
            d2 = sp.tile([128, ng], _F32)
            rmean = sp.tile([128, ng], _F32)
            rvar = sp.tile([128, ng], _F32)
            tmp = sp.tile([128, ng], _F32)
            std = sp.tile([128, ng], _F32)
            scal = sp.tile([128, ng], _F32)
            shif = sp.tile([128, ng], _F32)
            vt = sp.tile([128, ng], _F32)
            ivt = sp.tile([128, ng], _F32)
            r1 = sp.tile([128, ng], _F32)
            r2 = sp.tile([128, ng], _F32)
            siv = sp.tile([128, ng], _F32)
            klt = sp.tile([128, ng], _F32)

            # critical path to (scal, shif) first; KL afterwards
            vec.tensor_scalar_mul(mean[:], sums, 1.0 / N_GLOBAL)
            vec.tensor_scalar_mul(e2[:], sqs, 1.0 / N_GLOBAL)
            vec.tensor_mul(bvar[:], mean[:], mean[:])
            vec.tensor_sub(bvar[:], e2[:], bvar[:])
            # rm_t = 0.9*nrm + 0.1*mean ; rv_t = 0.9*nrv + (0.1*n/(n-1))*bvar
            vec.tensor_scalar_mul(rmt[:], mean[:], MOMENTUM)
            vec.tensor_add(rmt[:], rmt[:], cc(C_A1))
            vec.tensor_scalar_mul(rvt[:], bvar[:], MOMENTUM * N_GLOBAL / (N_GLOBAL - 1))
            vec.tensor_add(rvt[:], rvt[:], cc(C_A2))
            vec.tensor_sub(d[:], cc(C_LRM), rmt[:])
            vec.tensor_mul(d2[:], d[:], d[:])
            # running_mean = B1 + Q*rm_t ; running_var = B2 + Q*rv_t + R*d2
            vec.tensor_mul(rmean[:], cc(C_Q), rmt[:])
            vec.tensor_add(rmean[:], rmean[:], cc(C_B1))
            vec.tensor_mul(rvar[:], cc(C_Q), rvt[:])
            vec.tensor_add(rvar[:], rvar[:], cc(C_B2))
            vec.tensor_mul(tmp[:], cc(C_R), d2[:])
            vec.tensor_add(rvar[:], rvar[:], tmp[:])
            # scale = gamma / sqrt(running_var + eps); shift = beta - rmean*scale
            vec.tensor_scalar_add(rvar[:], rvar[:], EPS)
            act.activation(std[:], rvar[:], _AF.Sqrt, bias=0.0, scale=1.0)
            vec.reciprocal(std[:], std[:])
            vec.tensor_mul(scal[:], cc(C_GAM), std[:])
            vec.tensor_mul(shif[:], rmean[:], scal[:])
            vec.tensor_sub(shif[:], cc(C_BET), shif[:])

            # ---- pass 2: y = x*scale + shift ----
            for t in range(0 if "pass1" in variant else NT):
                g = t % ng
                if cache:
                    xt = xtiles[t]
                else:
                    xt = xpool.tile([128, FREE], xdt, tag="xt")
                    (nc.sync if t % 2 == 0 else nc.gpsimd).dma_start(xt[:], x[t])
                s_ap, b_ap = scal[:, g : g + 1], shif[:, g : g + 1]
                if xdt == ydt and not cache:
                    if t % 2 == 0:
                        act.activation(
                            xt[:], xt[:], _AF.Identity, bias=b_ap, scale=s_ap
                        )
                    else:
                        vec.tensor_scalar(
                            xt[:], xt[:], s_ap, b_ap, _ALU.mult, _ALU.add
                        )
                    (nc.sync if t % 2 == 0 else nc.gpsimd).dma_start(y[t], xt[:])
                else:
                    yt = ypool.tile([128, FREE], ydt, tag="yt")
                    if t % 2 == 0:
                        act.activation(
                            yt[:], xt[:], _AF.Identity, bias=b_ap, scale=s_ap
                        )
                    else:
                        vec.tensor_scalar(
                            yt[:], xt[:], s_ap, b_ap, _ALU.mult, _ALU.add
                        )
                    (nc.sync if t % 2 == 0 else nc.gpsimd).dma_start(y[t], yt[:])

            # ---- KL terms (off the critical path) ----
            # kl_c = 0.25*(vs/vt + vt/vs + d2*(1/vs + 1/vt) - 2)
            vec.tensor_scalar_add(vt[:], rvt[:], JITTER)
            vec.reciprocal(ivt[:], vt[:])
            vec.tensor_mul(r1[:], cc(C_VS), ivt[:])
            vec.tensor_mul(r2[:], vt[:], cc(C_IVS))
            vec.tensor_add(siv[:], cc(C_IVS), ivt[:])
            vec.tensor_mul(siv[:], d2[:], siv[:])
            vec.tensor_add(klt[:], r1[:], r2[:])
            vec.tensor_add(klt[:], klt[:], siv[:])
            vec.tensor_scalar(klt[:], klt[:], 0.25, -0.5, _ALU.mult, _ALU.add)
            # gpsimd (SWDGE), NOT sync: a sync-queue store here would
            # head-of-line-block pass-2 DMAs behind the finalize chain
            nc.gpsimd.dma_start(kl, klt[:klp, :])

    nc.compile()
    return nc


_PROGRAM_CACHE = {}


def _get_program(shard: str, in_dt: str, cache: bool, variant: str = "full",
                 out_dt: str = "float32"):
    key = (shard, in_dt, cache, variant, out_dt)
    if key not in _PROGRAM_CACHE:
        _PROGRAM_CACHE[key] = _build(shard, in_dt, cache, variant, out_dt)
    return _PROGRAM_CACHE[key]


def _const_cols(inputs, p):
    lrm = np.asarray(inputs["layer_running_mean"], np.float32)
    lrv = np.asarray(inputs["layer_running_var"], np.float32)
    gam = np.asarray(inputs["layer_weight"], np.float32)
    bet = np.asarray(inputs["layer_bias"], np.float32)
    nrm = np.asarray(inputs["norm_running_mean"], np.float32)
    nrv = np.asarray(inputs["norm_running_var"], np.float32)
    vs = lrv + np.float32(JITTER)
    cols = np.zeros((C, NCONST), np.float32)
    cols[:, C_A1] = (1.0 - MOMENTUM) * nrm
    cols[:, C_A2] = (1.0 - MOMENTUM) * nrv
    cols[:, C_B1] = p * lrm
    cols[:, C_B2] = p * lrv
    cols[:, C_LRM] = lrm
    cols[:, C_VS] = vs
    cols[:, C_IVS] = 1.0 / vs
    cols[:, C_GAM] = gam
    cols[:, C_BET] = bet
    cols[:, C_Q] = 1.0 - p
    cols[:, C_R] = p * (1.0 - p)
    return cols


def _prepare_in_maps(inputs, shard, in_dt):
    x = np.asarray(inputs["input"], np.float32)
    assert x.shape == (N, C, H, W), x.shape
    p = float(np.asarray(inputs["prior"], np.float32)[0])
    cols = _const_cols(inputs, p)
    xdt = np.float16 if in_dt == "float16" else np.float32
    xr = x.reshape(N, C, FREE)
    in_maps = []
    if shard == "channel":
        ii = np.arange(128)
        sel = (ii[:, None] % CPC == ii[None, :] % CPC).astype(np.float32)
        for k in range(N_CORES):
            ck = slice(k * CPC, (k + 1) * CPC)
            # [64, 32, F] -> tiles [16, (quarter, channel)=128, F]
            xs = (
                xr[:, ck, :]
                .reshape(NQ, NT, CPC, FREE)
                .transpose(1, 0, 2, 3)
                .reshape(NT, 128, FREE)
            )
            in_maps.append({
                "x": np.ascontiguousarray(xs, dtype=xdt),
                "cvec": np.ascontiguousarray(
                    np.tile(cols[ck], (NQ, 1))[:, None, :]
                ),
                "sel": sel,
            })
    else:
        consts = np.ascontiguousarray(
            cols.reshape(2, 128, NCONST).transpose(1, 0, 2)
        )
        for k in range(N_CORES):
            shard_x = xr[k * NB : (k + 1) * NB].reshape(NT, 128, FREE)
            in_maps.append({
                "x": np.ascontiguousarray(shard_x, dtype=xdt),
                "cvec": consts,
            })
    return in_maps


def _assemble_out(shard, per_core_y, per_core_kl):
    out = np.empty((N, C, FREE), np.float32)
    if shard == "channel":
        for k in range(N_CORES):
            yk = np.asarray(per_core_y[k]).reshape(NT, NQ, CPC, FREE)
            out[:, k * CPC : (k + 1) * CPC, :] = (
                yk.transpose(1, 0, 2, 3).reshape(N, CPC, FREE)
            )
        div = np.float32(
            sum(np.asarray(klk, np.float64).sum() for klk in per_core_kl)
        )
    else:
        for k in range(N_CORES):
            out[k * NB : (k + 1) * NB] = np.asarray(per_core_y[k]).reshape(
                NB, C, FREE
            )
        div = np.float32(np.asarray(per_core_kl[0], np.float64).sum())
    return out.reshape(N, C, H, W), div


def kernel(**inputs):
    shard, in_dt, cache = CONFIG["shard"], CONFIG["in_dt"], CONFIG["cache"]
    in_maps = _prepare_in_maps(inputs, shard, in_dt)
    nc = _get_program(shard, in_dt, cache, out_dt=CONFIG.get("out_dt", "float32"))

    old_m = nc.m
    nc.m = get_hw_module(nc.m)
    try:
        res = bass_utils.run_bass_kernel_spmd(nc, in_maps, core_ids=list(range(N_CORES)))
    finally:
        nc.m = old_m

    return _assemble_out(
        shard,
        [res.results[k]["y"] for k in range(N_CORES)],
        [res.results[k]["kl"] for k in range(N_CORES)],
    )


# revision 26
# speedup vs baseline: 1.2921x; 1.2921x over previous
"""BayesianBatchNorm on 8 TRN2 NeuronCores.

Two sharding strategies:

"channel" (default): each core owns 32 channels across the FULL batch, so
  the batch statistics complete locally — no cross-core communication.
  Host lays the core's data out as 16 tiles of [128, 3136] where
  partition p = (batch_quarter, channel) = (p//32, p%32); a single
  TensorE matmul against a 0/1 selection matrix reduces the per-quarter
  partial sums across partitions AND broadcasts the result back to all
  128 partitions. The KL term for the core's 32 channels ships out and
  the host sums the 8x32 terms into the div scalar.

"batch": data-parallel over batch; per-channel (sum, sumsq) partials are
  AllReduced across cores (2KB payload). Kept for comparison — the
  collective costs 25-55us wall on this environment.

Pass 2 normalizes y = x*scale + shift per channel and streams out.
"""

import sys

sys.path.insert(0, "/opt/trn_rl_repo")

import numpy as np

from concourse import bass, mybir
import concourse.bacc as bacc
import concourse.tile as tile
from concourse import bass_utils
from concourse.bass_interp import get_hw_module

N_CORES = 8
N, C, H, W = 64, 256, 56, 56
FREE = H * W                     # 3136
NT = 16                          # tiles of [128, FREE] per core
CPC = C // N_CORES               # 32 channels per core (channel mode)
NQ = 128 // CPC                  # 4 batch quarters on partitions (channel mode)
NB = N // N_CORES                # 8 batches per core (batch mode)
N_GLOBAL = N * FREE              # 200704 elements per channel
MOMENTUM = 0.1
EPS = 1e-5
JITTER = 1e-5

# consts tensor column layout
(C_A1, C_A2, C_B1, C_B2, C_LRM, C_VS, C_IVS, C_GAM, C_BET, C_Q, C_R) = range(11)
NCONST = 11

# shard: "channel" (no collective) or "batch" (AllReduce).
# in_dt: dtype x is stored in DRAM as (host casts).
# cache: keep x tiles resident in SBUF between the two passes (skips the
# second HBM read of x).
CONFIG = dict(shard="channel", in_dt="float16", cache=True, out_dt="float32")

_ALU = mybir.AluOpType
_AF = mybir.ActivationFunctionType
_F32 = mybir.dt.float32


def _build(shard: str, in_dt: str, cache: bool, variant: str = "full",
           out_dt: str = "float32", dma: str = "sync_gpsimd"):
    ng = 2 if shard == "batch" else 1
    nc = bacc.Bacc("TRN2", debug=False, enable_asserts=False, num_devices=N_CORES)
    xdt = mybir.dt.float16 if in_dt == "float16" else _F32
    ydt = mybir.dt.float16 if out_dt == "float16" else _F32

    x = nc.dram_tensor("x", [NT, 128, FREE], xdt, kind="ExternalInput").ap()
    cvec = nc.dram_tensor("cvec", [128, ng, NCONST], _F32, kind="ExternalInput").ap()
    y = nc.dram_tensor("y", [NT, 128, FREE], ydt, kind="ExternalOutput").ap()
    klp = CPC if shard == "channel" else 128
    kl = nc.dram_tensor("kl", [klp, ng], _F32, kind="ExternalOutput").ap()
    if shard == "channel":
        sel = nc.dram_tensor("sel", [128, 128], _F32, kind="ExternalInput").ap()

    vec = nc.vector
    act = nc.scalar
    odd_eng = {"sync": nc.sync, "sync_scalar": nc.scalar, "sync_gpsimd": nc.gpsimd}[dma]

    with tile.TileContext(nc) as tc:
        with (
            tc.tile_pool(name="xin", bufs=NT if cache else 6) as xpool,
            tc.tile_pool(name="yout", bufs=4) as ypool,
            tc.tile_pool(name="small", bufs=1) as sp,
            tc.tile_pool(name="psum", bufs=1, space="PSUM") as pp,
            tc.tile_pool(name="dram", bufs=1, space="DRAM") as dp,
        ):
            cv = sp.tile([128, ng, NCONST], _F32)
            nc.sync.dma_start(cv[:], cvec)
            if shard == "channel":
                sel_t = sp.tile([128, 128], _F32)
                nc.sync.dma_start(sel_t[:], sel)

            def cc(k):  # [128, ng] column view of the consts
                return cv[:, :, k]

            # ---- pass 1: load tiles, per-tile (sum, sumsq) partials ----
            npart = NT // ng
            sum_part = [
                sp.tile([128, npart], _F32, tag=f"sp{g}", name=f"sum_part{g}")
                for g in range(ng)
            ]
            sq_part = [
                sp.tile([128, npart], _F32, tag=f"qp{g}", name=f"sq_part{g}")
                for g in range(ng)
            ]
            xtiles = []
            for t in range(NT):
                g, nn = t % ng, t // ng
                xt = xpool.tile([128, FREE], xdt, tag="xt")
                (nc.sync if t % 2 == 0 else odd_eng).dma_start(xt[:], x[t])
                vec.reduce_sum(
                    sum_part[g][:, nn : nn + 1], xt[:], axis=mybir.AxisListType.X
                )
                scr = sp.tile(
                    [128, FREE], mybir.dt.float16, tag="scr", bufs=2, name="scr"
                )
                act.activation(
                    scr[:],
                    xt[:],
                    _AF.Square,
                    bias=0.0,
                    scale=1.0,
                    accum_out=sq_part[g][:, nn : nn + 1],
                )
                xtiles.append(xt)

            # ---- combine partials into global per-channel (sum, sumsq) ----
            cc_out = sp.tile([128, 2 * ng], _F32)
            if shard == "channel":
                packed = sp.tile([128, 2], _F32)
                vec.reduce_sum(packed[:, 0:1], sum_part[0][:], axis=mybir.AxisListType.X)
                vec.reduce_sum(packed[:, 1:2], sq_part[0][:], axis=mybir.AxisListType.X)
                # one matmul: sel[k,m]=1 iff k%32==m%32 reduces the 4 batch
                # quarters AND broadcasts back to all 128 partitions
                ps = pp.tile([128, 2], _F32)
                nc.tensor.matmul(ps[:], sel_t[:], packed[:], start=True, stop=True)
                vec.tensor_copy(cc_out[:], ps[:])
            else:
                cc_in = sp.tile([128, 2 * ng], _F32)
                for g in range(ng):
                    vec.reduce_sum(
                        cc_in[:, g : g + 1], sum_part[g][:], axis=mybir.AxisListType.X
                    )
                    vec.reduce_sum(
                        cc_in[:, ng + g : ng + g + 1],
                        sq_part[g][:],
                        axis=mybir.AxisListType.X,
                    )
                bounce_in = dp.tile([128, 2 * ng], _F32)
                bounce_out = dp.tile([128, 2 * ng], _F32)
                nc.gpsimd.dma_start(bounce_in[:], cc_in[:])
                if "nocc" in variant:
                    nc.gpsimd.dma_start(bounce_out[:], bounce_in[:])
                else:
                    nc.gpsimd.collective_compute(
                        "AllReduce",
                        _ALU.add,
                        replica_groups=[list(range(N_CORES))],
                        ins=[bounce_in.opt()],
                        outs=[bounce_out.opt()],
                    )
                nc.gpsimd.dma_start(cc_out[:], bounce_out[:])

            # ---- finalize: all [128, ng] elementwise ----
            sums, sqs = cc_out[:, 0:ng], cc_out[:, ng : 2 * ng]
            mean = sp.tile([128, ng], _F32)
            e2 = sp.tile([128, ng], _F32)
            bvar = sp.tile([128, ng], _F32)
            rmt = sp.tile([128, ng], _F32)
            rvt = sp.tile([128, ng], _F32)
            d = sp.tile([128, ng], _F32)
            d2 = sp.tile([128, ng], _F32)
            rmean = sp.tile([128, ng], _F32)
            rvar = sp.tile([128, ng], _F32)
            tmp = sp.tile([128, ng], _F32)
            std = sp.tile([128, ng], _F32)
            scal = sp.tile([128, ng], _F32)
            shif = sp.tile([128, ng], _F32)
            vt = sp.tile([128, ng], _F32)
            ivt = sp.tile([128, ng], _F32)
            r1 = sp.tile([128, ng], _F32)
            r2 = sp.tile([128, ng], _F32)
            siv = sp.tile([128, ng], _F32)
            klt = sp.tile([128, ng], _F32)

            # critical path to (scal, shif) first; KL afterwards
            vec.tensor_scalar_mul(mean[:], sums, 1.0 / N_GLOBAL)
            vec.tensor_scalar_mul(e2[:], sqs, 1.0 / N_GLOBAL)
            vec.tensor_mul(bvar[:], mean[:], mean[:])
            vec.tensor_sub(bvar[:], e2[:], bvar[:])
            # rm_t = 0.9*nrm + 0.1*mean ; rv_t = 0.9*nrv + (0.1*n/(n-1))*bvar
            vec.tensor_scalar_mul(rmt[:], mean[:], MOMENTUM)
            vec.tensor_add(rmt[:], rmt[:], cc(C_A1))
            vec.tensor_scalar_mul(rvt[:], bvar[:], MOMENTUM * N_GLOBAL / (N_GLOBAL - 1))
            vec.tensor_add(rvt[:], rvt[:], cc(C_A2))
            vec.tensor_sub(d[:], cc(C_LRM), rmt[:])
            vec.tensor_mul(d2[:], d[:], d[:])
            # running_mean = B1 + Q*rm_t ; running_var = B2 + Q*rv_t + R*d2
            vec.tensor_mul(rmean[:], cc(C_Q), rmt[:])
            vec.tensor_add(rmean[:], rmean[:], cc(C_B1))
            vec.tensor_mul(rvar[:], cc(C_Q), rvt[:])
            vec.tensor_add(rvar[:], rvar[:], cc(C_B2))
            vec.tensor_mul(tmp[:], cc(C_R), d2[:])
            vec.tensor_add(rvar[:], rvar[:], tmp[:])
            # scale = gamma / sqrt(running_var + eps); shift = beta - rmean*scale
            vec.tensor_scalar_add(rvar[:], rvar[:], EPS)
            act.activation(std[:], rvar[:], _AF.Sqrt, bias=0.0, scale=1.0)
            vec.reciprocal(std[:], std[:])
            vec.tensor_mul(scal[:], cc(C_GAM), std[:])
            vec.tensor_mul(shif[:], rmean[:], scal[:])
            vec.tensor_sub(shif[:], cc(C_BET), shif[:])

            # ---- pass 2: y = x*scale + shift ----
            for t in range(0 if "pass1" in variant else NT):
                g = t % ng
                if cache:
                    xt = xtiles[t]
                else:
                    xt = xpool.tile([128, FREE], xdt, tag="xt")
                    (nc.sync if t % 2 == 0 else odd_eng).dma_start(xt[:], x[t])
                s_ap, b_ap = scal[:, g : g + 1], shif[:, g : g + 1]
                if xdt == ydt and not cache:
                    if t % 2 == 0:
                        act.activation(
                            xt[:], xt[:], _AF.Identity, bias=b_ap, scale=s_ap
                        )
                    else:
                        vec.tensor_scalar(
                            xt[:], xt[:], s_ap, b_ap, _ALU.mult, _ALU.add
                        )
                    (nc.sync if t % 2 == 0 else odd_eng).dma_start(y[t], xt[:])
                else:
                    yt = ypool.tile([128, FREE], ydt, tag="yt")
                    if t % 2 == 0:
                        act.activation(
                            yt[:], xt[:], _AF.Identity, bias=b_ap, scale=s_ap
                        )
                    else:
                        vec.tensor_scalar(
                            yt[:], xt[:], s_ap, b_ap, _ALU.mult, _ALU.add
                        )
                    (nc.sync if t % 2 == 0 else odd_eng).dma_start(y[t], yt[:])

            # ---- KL terms (off the critical path) ----
            # kl_c = 0.25*(vs/vt + vt/vs + d2*(1/vs + 1/vt) - 2)
            vec.tensor_scalar_add(vt[:], rvt[:], JITTER)
            vec.reciprocal(ivt[:], vt[:])
            vec.tensor_mul(r1[:], cc(C_VS), ivt[:])
            vec.tensor_mul(r2[:], vt[:], cc(C_IVS))
            vec.tensor_add(siv[:], cc(C_IVS), ivt[:])
            vec.tensor_mul(siv[:], d2[:], siv[:])
            vec.tensor_add(klt[:], r1[:], r2[:])
            vec.tensor_add(klt[:], klt[:], siv[:])
            vec.tensor_scalar(klt[:], klt[:], 0.25, -0.5, _ALU.mult, _ALU.add)
            # gpsimd (SWDGE), NOT sync: a sync-queue store here would
            # head-of-line-block pass-2 DMAs behind the finalize chain
            nc.gpsimd.dma_start(kl, klt[:klp, :])

    nc.compile()
    return nc


_PROGRAM_CACHE = {}


def _get_program(shard: str, in_dt: str, cache: bool, variant: str = "full",
                 out_dt: str = "float32", dma: str = "sync_gpsimd"):
    key = (shard, in_dt, cache, variant, out_dt, dma)
    if key not in _PROGRAM_CACHE:
        _PROGRAM_CACHE[key] = _build(shard, in_dt, cache, variant, out_dt, dma)
    return _PROGRAM_CACHE[key]


def _const_cols(inputs, p):
    lrm = np.asarray(inputs["layer_running_mean"], np.float32)
    lrv = np.asarray(inputs["layer_running_var"], np.float32)
    gam = np.asarray(inputs["layer_weight"], np.float32)
    bet = np.asarray(inputs["layer_bias"], np.float32)
    nrm = np.asarray(inputs["norm_running_mean"], np.float32)
    nrv = np.asarray(inputs["norm_running_var"], np.float32)
    vs = lrv + np.float32(JITTER)
    cols = np.zeros((C, NCONST), np.float32)
    cols[:, C_A1] = (1.0 - MOMENTUM) * nrm
    cols[:, C_A2] = (1.0 - MOMENTUM) * nrv
    cols[:, C_B1] = p * lrm
    cols[:, C_B2] = p * lrv
    cols[:, C_LRM] = lrm
    cols[:, C_VS] = vs
    cols[:, C_IVS] = 1.0 / vs
    cols[:, C_GAM] = gam
    cols[:, C_BET] = bet
    cols[:, C_Q] = 1.0 - p
    cols[:, C_R] = p * (1.0 - p)
    return cols


def _prepare_in_maps(inputs, shard, in_dt):
    x = np.asarray(inputs["input"], np.float32)
    assert x.shape == (N, C, H, W), x.shape
    p = float(np.asarray(inputs["prior"], np.float32)[0])
    cols = _const_cols(inputs, p)
    xdt = np.float16 if in_dt == "float16" else np.float32
    xr = x.reshape(N, C, FREE)
    in_maps = []
    if shard == "channel":
        ii = np.arange(128)
        sel = (ii[:, None] % CPC == ii[None, :] % CPC).astype(np.float32)
        for k in range(N_CORES):
            ck = slice(k * CPC, (k + 1) * CPC)
            # [64, 32, F] -> tiles [16, (quarter, channel)=128, F]
            xs = (
                xr[:, ck, :]
                .reshape(NQ, NT, CPC, FREE)
                .transpose(1, 0, 2, 3)
                .reshape(NT, 128, FREE)
            )
            in_maps.append({
                "x": np.ascontiguousarray(xs, dtype=xdt),
                "cvec": np.ascontiguousarray(
                    np.tile(cols[ck], (NQ, 1))[:, None, :]
                ),
                "sel": sel,
            })
    else:
        consts = np.ascontiguousarray(
            cols.reshape(2, 128, NCONST).transpose(1, 0, 2)
        )
        for k in range(N_CORES):
            shard_x = xr[k * NB : (k + 1) * NB].reshape(NT, 128, FREE)
            in_maps.append({
                "x": np.ascontiguousarray(shard_x, dtype=xdt),
                "cvec": consts,
            })
    return in_maps


def _assemble_out(shard, per_core_y, per_core_kl):
    out = np.empty((N, C, FREE), np.float32)
    if shard == "channel":
        for k in range(N_CORES):
            yk = np.asarray(per_core_y[k]).reshape(NT, NQ, CPC, FREE)
            out[:, k * CPC : (k + 1) * CPC, :] = (
                yk.transpose(1, 0, 2, 3).reshape(N, CPC, FREE)
            )
        div = np.float32(
            sum(np.asarray(klk, np.float64).sum() for klk in per_core_kl)
        )
    else:
        for k in range(N_CORES):
            out[k * NB : (k + 1) * NB] = np.asarray(per_core_y[k]).reshape(
                NB, C, FREE
            )
        div = np.float32(np.asarray(per_core_kl[0], np.float64).sum())
    return out.reshape(N, C, H, W), div


def kernel(**inputs):
    shard, in_dt, cache = CONFIG["shard"], CONFIG["in_dt"], CONFIG["cache"]
    in_maps = _prepare_in_maps(inputs, shard, in_dt)
    nc = _get_program(shard, in_dt, cache, out_dt=CONFIG.get("out_dt", "float32"),
                      dma=CONFIG.get("dma", "sync_gpsimd"))

    old_m = nc.m
    nc.m = get_hw_module(nc.m)
    try:
        res = bass_utils.run_bass_kernel_spmd(nc, in_maps, core_ids=list(range(N_CORES)))
    finally:
        nc.m = old_m

    return _assemble_out(
        shard,
        [res.results[k]["y"] for k in range(N_CORES)],
        [res.results[k]["kl"] for k in range(N_CORES)],
    )


# revision 27
# speedup vs baseline: 1.3437x; 1.0399x over previous
"""BayesianBatchNorm on 8 TRN2 NeuronCores.

Two sharding strategies:

"channel" (default): each core owns 32 channels across the FULL batch, so
  the batch statistics complete locally — no cross-core communication.
  Host lays the core's data out as 16 tiles of [128, 3136] where
  partition p = (batch_quarter, channel) = (p//32, p%32); a single
  TensorE matmul against a 0/1 selection matrix reduces the per-quarter
  partial sums across partitions AND broadcasts the result back to all
  128 partitions. The KL term for the core's 32 channels ships out and
  the host sums the 8x32 terms into the div scalar.

"batch": data-parallel over batch; per-channel (sum, sumsq) partials are
  AllReduced across cores (2KB payload). Kept for comparison — the
  collective costs 25-55us wall on this environment.

Pass 2 normalizes y = x*scale + shift per channel and streams out.
"""

import sys

sys.path.insert(0, "/opt/trn_rl_repo")

import numpy as np

from concourse import bass, mybir
import concourse.bacc as bacc
import concourse.tile as tile
from concourse import bass_utils
from concourse.bass_interp import get_hw_module

N_CORES = 8
N, C, H, W = 64, 256, 56, 56
FREE = H * W                     # 3136
NT = 16                          # tiles of [128, FREE] per core
CPC = C // N_CORES               # 32 channels per core (channel mode)
NQ = 128 // CPC                  # 4 batch quarters on partitions (channel mode)
NB = N // N_CORES                # 8 batches per core (batch mode)
N_GLOBAL = N * FREE              # 200704 elements per channel
MOMENTUM = 0.1
EPS = 1e-5
JITTER = 1e-5

# consts tensor column layout
(C_A1, C_A2, C_B1, C_B2, C_LRM, C_VS, C_IVS, C_GAM, C_BET, C_Q, C_R) = range(11)
NCONST = 11

# shard: "channel" (no collective) or "batch" (AllReduce).
# in_dt: dtype x is stored in DRAM as (host casts).
# cache: keep x tiles resident in SBUF between the two passes (skips the
# second HBM read of x).
CONFIG = dict(shard="channel", in_dt="float16", cache=True, out_dt="float32", tpb=2)

_ALU = mybir.AluOpType
_AF = mybir.ActivationFunctionType
_F32 = mybir.dt.float32


def _build(shard: str, in_dt: str, cache: bool, variant: str = "full",
           out_dt: str = "float32", dma: str = "sync_gpsimd", tpb: int = 1):
    ng = 2 if shard == "batch" else 1
    assert tpb == 1 or (shard == "channel" and NT % tpb == 0)
    nt = NT // tpb
    fr = FREE * tpb
    nc = bacc.Bacc("TRN2", debug=False, enable_asserts=False, num_devices=N_CORES)
    xdt = mybir.dt.float16 if in_dt == "float16" else _F32
    ydt = mybir.dt.float16 if out_dt == "float16" else _F32

    x = nc.dram_tensor("x", [nt, 128, fr], xdt, kind="ExternalInput").ap()
    cvec = nc.dram_tensor("cvec", [128, ng, NCONST], _F32, kind="ExternalInput").ap()
    y = nc.dram_tensor("y", [nt, 128, fr], ydt, kind="ExternalOutput").ap()
    klp = CPC if shard == "channel" else 128
    kl = nc.dram_tensor("kl", [klp, ng], _F32, kind="ExternalOutput").ap()
    if shard == "channel":
        sel = nc.dram_tensor("sel", [128, 128], _F32, kind="ExternalInput").ap()

    vec = nc.vector
    act = nc.scalar
    odd_eng = {"sync": nc.sync, "sync_scalar": nc.scalar, "sync_gpsimd": nc.gpsimd}[dma]

    with tile.TileContext(nc) as tc:
        with (
            tc.tile_pool(name="xin", bufs=nt if cache else 6) as xpool,
            tc.tile_pool(name="yout", bufs=4 if out_dt == "float16" or tpb == 1 else 2) as ypool,
            tc.tile_pool(name="small", bufs=1) as sp,
            tc.tile_pool(name="psum", bufs=1, space="PSUM") as pp,
            tc.tile_pool(name="dram", bufs=1, space="DRAM") as dp,
        ):
            cv = sp.tile([128, ng, NCONST], _F32)
            nc.gpsimd.dma_start(cv[:], cvec)
            if shard == "channel":
                sel_t = sp.tile([128, 128], _F32)
                nc.gpsimd.dma_start(sel_t[:], sel)

            def cc(k):  # [128, ng] column view of the consts
                return cv[:, :, k]

            # ---- pass 1: load tiles, per-tile (sum, sumsq) partials ----
            npart = nt // ng
            sum_part = [
                sp.tile([128, npart], _F32, tag=f"sp{g}", name=f"sum_part{g}")
                for g in range(ng)
            ]
            sq_part = [
                sp.tile([128, npart], _F32, tag=f"qp{g}", name=f"sq_part{g}")
                for g in range(ng)
            ]
            xtiles = []
            for t in range(nt):
                g, nn = t % ng, t // ng
                xt = xpool.tile([128, fr], xdt, tag="xt")
                (nc.sync if t % 2 == 0 else odd_eng).dma_start(xt[:], x[t])
                vec.reduce_sum(
                    sum_part[g][:, nn : nn + 1], xt[:], axis=mybir.AxisListType.X
                )
                scr = sp.tile(
                    [128, fr], mybir.dt.float16, tag="scr", bufs=2, name="scr"
                )
                act.activation(
                    scr[:],
                    xt[:],
                    _AF.Square,
                    bias=0.0,
                    scale=1.0,
                    accum_out=sq_part[g][:, nn : nn + 1],
                )
                xtiles.append(xt)

            # ---- combine partials into global per-channel (sum, sumsq) ----
            cc_out = sp.tile([128, 2 * ng], _F32)
            if shard == "channel":
                packed = sp.tile([128, 2], _F32)
                vec.reduce_sum(packed[:, 0:1], sum_part[0][:], axis=mybir.AxisListType.X)
                vec.reduce_sum(packed[:, 1:2], sq_part[0][:], axis=mybir.AxisListType.X)
                # one matmul: sel[k,m]=1 iff k%32==m%32 reduces the 4 batch
                # quarters AND broadcasts back to all 128 partitions
                ps = pp.tile([128, 2], _F32)
                nc.tensor.matmul(ps[:], sel_t[:], packed[:], start=True, stop=True)
                vec.tensor_copy(cc_out[:], ps[:])
            else:
                cc_in = sp.tile([128, 2 * ng], _F32)
                for g in range(ng):
                    vec.reduce_sum(
                        cc_in[:, g : g + 1], sum_part[g][:], axis=mybir.AxisListType.X
                    )
                    vec.reduce_sum(
                        cc_in[:, ng + g : ng + g + 1],
                        sq_part[g][:],
                        axis=mybir.AxisListType.X,
                    )
                bounce_in = dp.tile([128, 2 * ng], _F32)
                bounce_out = dp.tile([128, 2 * ng], _F32)
                nc.gpsimd.dma_start(bounce_in[:], cc_in[:])
                if "nocc" in variant:
                    nc.gpsimd.dma_start(bounce_out[:], bounce_in[:])
                else:
                    nc.gpsimd.collective_compute(
                        "AllReduce",
                        _ALU.add,
                        replica_groups=[list(range(N_CORES))],
                        ins=[bounce_in.opt()],
                        outs=[bounce_out.opt()],
                    )
                nc.gpsimd.dma_start(cc_out[:], bounce_out[:])

            # ---- finalize: all [128, ng] elementwise ----
            sums, sqs = cc_out[:, 0:ng], cc_out[:, ng : 2 * ng]
            mean = sp.tile([128, ng], _F32)
            e2 = sp.tile([128, ng], _F32)
            bvar = sp.tile([128, ng], _F32)
            rmt = sp.tile([128, ng], _F32)
            rvt = sp.tile([128, ng], _F32)
            d = sp.tile([128, ng], _F32)
            d2 = sp.tile([128, ng], _F32)
            rmean = sp.tile([128, ng], _F32)
            rvar = sp.tile([128, ng], _F32)
            tmp = sp.tile([128, ng], _F32)
            std = sp.tile([128, ng], _F32)
            scal = sp.tile([128, ng], _F32)
            shif = sp.tile([128, ng], _F32)
            vt = sp.tile([128, ng], _F32)
            ivt = sp.tile([128, ng], _F32)
            r1 = sp.tile([128, ng], _F32)
            r2 = sp.tile([128, ng], _F32)
            siv = sp.tile([128, ng], _F32)
            klt = sp.tile([128, ng], _F32)

            # critical path to (scal, shif) first; KL afterwards
            vec.tensor_scalar_mul(mean[:], sums, 1.0 / N_GLOBAL)
            vec.tensor_scalar_mul(e2[:], sqs, 1.0 / N_GLOBAL)
            vec.tensor_mul(bvar[:], mean[:], mean[:])
            vec.tensor_sub(bvar[:], e2[:], bvar[:])
            # rm_t = 0.9*nrm + 0.1*mean ; rv_t = 0.9*nrv + (0.1*n/(n-1))*bvar
            vec.tensor_scalar_mul(rmt[:], mean[:], MOMENTUM)
            vec.tensor_add(rmt[:], rmt[:], cc(C_A1))
            vec.tensor_scalar_mul(rvt[:], bvar[:], MOMENTUM * N_GLOBAL / (N_GLOBAL - 1))
            vec.tensor_add(rvt[:], rvt[:], cc(C_A2))
            vec.tensor_sub(d[:], cc(C_LRM), rmt[:])
            vec.tensor_mul(d2[:], d[:], d[:])
            # running_mean = B1 + Q*rm_t ; running_var = B2 + Q*rv_t + R*d2
            vec.tensor_mul(rmean[:], cc(C_Q), rmt[:])
            vec.tensor_add(rmean[:], rmean[:], cc(C_B1))
            vec.tensor_mul(rvar[:], cc(C_Q), rvt[:])
            vec.tensor_add(rvar[:], rvar[:], cc(C_B2))
            vec.tensor_mul(tmp[:], cc(C_R), d2[:])
            vec.tensor_add(rvar[:], rvar[:], tmp[:])
            # scale = gamma / sqrt(running_var + eps); shift = beta - rmean*scale
            vec.tensor_scalar_add(rvar[:], rvar[:], EPS)
            act.activation(std[:], rvar[:], _AF.Sqrt, bias=0.0, scale=1.0)
            vec.reciprocal(std[:], std[:])
            vec.tensor_mul(scal[:], cc(C_GAM), std[:])
            vec.tensor_mul(shif[:], rmean[:], scal[:])
            vec.tensor_sub(shif[:], cc(C_BET), shif[:])

            # ---- pass 2: y = x*scale + shift ----
            for t in range(0 if "pass1" in variant else nt):
                g = t % ng
                if cache:
                    xt = xtiles[t]
                else:
                    xt = xpool.tile([128, fr], xdt, tag="xt")
                    (nc.sync if t % 2 == 0 else odd_eng).dma_start(xt[:], x[t])
                s_ap, b_ap = scal[:, g : g + 1], shif[:, g : g + 1]
                if xdt == ydt and not cache:
                    if t % 2 == 0:
                        act.activation(
                            xt[:], xt[:], _AF.Identity, bias=b_ap, scale=s_ap
                        )
                    else:
                        vec.tensor_scalar(
                            xt[:], xt[:], s_ap, b_ap, _ALU.mult, _ALU.add
                        )
                    (nc.sync if t % 2 == 0 else odd_eng).dma_start(y[t], xt[:])
                else:
                    yt = ypool.tile([128, fr], ydt, tag="yt")
                    if t % 2 == 0:
                        act.activation(
                            yt[:], xt[:], _AF.Identity, bias=b_ap, scale=s_ap
                        )
                    else:
                        vec.tensor_scalar(
                            yt[:], xt[:], s_ap, b_ap, _ALU.mult, _ALU.add
                        )
                    (nc.sync if t % 2 == 0 else odd_eng).dma_start(y[t], yt[:])

            # ---- KL terms (off the critical path) ----
            # kl_c = 0.25*(vs/vt + vt/vs + d2*(1/vs + 1/vt) - 2)
            vec.tensor_scalar_add(vt[:], rvt[:], JITTER)
            vec.reciprocal(ivt[:], vt[:])
            vec.tensor_mul(r1[:], cc(C_VS), ivt[:])
            vec.tensor_mul(r2[:], vt[:], cc(C_IVS))
            vec.tensor_add(siv[:], cc(C_IVS), ivt[:])
            vec.tensor_mul(siv[:], d2[:], siv[:])
            vec.tensor_add(klt[:], r1[:], r2[:])
            vec.tensor_add(klt[:], klt[:], siv[:])
            vec.tensor_scalar(klt[:], klt[:], 0.25, -0.5, _ALU.mult, _ALU.add)
            # gpsimd (SWDGE), NOT sync: a sync-queue store here would
            # head-of-line-block pass-2 DMAs behind the finalize chain
            nc.gpsimd.dma_start(kl, klt[:klp, :])

    nc.compile()
    return nc


_PROGRAM_CACHE = {}


def _get_program(shard: str, in_dt: str, cache: bool, variant: str = "full",
                 out_dt: str = "float32", dma: str = "sync_gpsimd", tpb: int = 1):
    key = (shard, in_dt, cache, variant, out_dt, dma, tpb)
    if key not in _PROGRAM_CACHE:
        _PROGRAM_CACHE[key] = _build(shard, in_dt, cache, variant, out_dt, dma, tpb)
    return _PROGRAM_CACHE[key]


def _const_cols(inputs, p):
    lrm = np.asarray(inputs["layer_running_mean"], np.float32)
    lrv = np.asarray(inputs["layer_running_var"], np.float32)
    gam = np.asarray(inputs["layer_weight"], np.float32)
    bet = np.asarray(inputs["layer_bias"], np.float32)
    nrm = np.asarray(inputs["norm_running_mean"], np.float32)
    nrv = np.asarray(inputs["norm_running_var"], np.float32)
    vs = lrv + np.float32(JITTER)
    cols = np.zeros((C, NCONST), np.float32)
    cols[:, C_A1] = (1.0 - MOMENTUM) * nrm
    cols[:, C_A2] = (1.0 - MOMENTUM) * nrv
    cols[:, C_B1] = p * lrm
    cols[:, C_B2] = p * lrv
    cols[:, C_LRM] = lrm
    cols[:, C_VS] = vs
    cols[:, C_IVS] = 1.0 / vs
    cols[:, C_GAM] = gam
    cols[:, C_BET] = bet
    cols[:, C_Q] = 1.0 - p
    cols[:, C_R] = p * (1.0 - p)
    return cols


def _prepare_in_maps(inputs, shard, in_dt):
    x = np.asarray(inputs["input"], np.float32)
    assert x.shape == (N, C, H, W), x.shape
    p = float(np.asarray(inputs["prior"], np.float32)[0])
    cols = _const_cols(inputs, p)
    xdt = np.float16 if in_dt == "float16" else np.float32
    xr = x.reshape(N, C, FREE)
    in_maps = []
    if shard == "channel":
        tpb = int(CONFIG.get("tpb", 1))
        nt = NT // tpb
        ii = np.arange(128)
        sel = (ii[:, None] % CPC == ii[None, :] % CPC).astype(np.float32)
        for k in range(N_CORES):
            ck = slice(k * CPC, (k + 1) * CPC)
            # [64, 32, F] -> tiles [nt, (quarter, channel)=128, tpb*F]
            xs = (
                xr[:, ck, :]
                .reshape(NQ, nt, tpb, CPC, FREE)
                .transpose(1, 0, 3, 2, 4)
                .reshape(nt, 128, tpb * FREE)
            )
            in_maps.append({
                "x": np.ascontiguousarray(xs, dtype=xdt),
                "cvec": np.ascontiguousarray(
                    np.tile(cols[ck], (NQ, 1))[:, None, :]
                ),
                "sel": sel,
            })
    else:
        consts = np.ascontiguousarray(
            cols.reshape(2, 128, NCONST).transpose(1, 0, 2)
        )
        for k in range(N_CORES):
            shard_x = xr[k * NB : (k + 1) * NB].reshape(NT, 128, FREE)
            in_maps.append({
                "x": np.ascontiguousarray(shard_x, dtype=xdt),
                "cvec": consts,
            })
    return in_maps


def _assemble_out(shard, per_core_y, per_core_kl):
    out = np.empty((N, C, FREE), np.float32)
    if shard == "channel":
        tpb = int(CONFIG.get("tpb", 1))
        nt = NT // tpb
        for k in range(N_CORES):
            yk = np.asarray(per_core_y[k]).reshape(nt, NQ, CPC, tpb, FREE)
            out[:, k * CPC : (k + 1) * CPC, :] = (
                yk.transpose(1, 0, 3, 2, 4).reshape(N, CPC, FREE)
            )
        div = np.float32(
            sum(np.asarray(klk, np.float64).sum() for klk in per_core_kl)
        )
    else:
        for k in range(N_CORES):
            out[k * NB : (k + 1) * NB] = np.asarray(per_core_y[k]).reshape(
                NB, C, FREE
            )
        div = np.float32(np.asarray(per_core_kl[0], np.float64).sum())
    return out.reshape(N, C, H, W), div


def kernel(**inputs):
    shard, in_dt, cache = CONFIG["shard"], CONFIG["in_dt"], CONFIG["cache"]
    in_maps = _prepare_in_maps(inputs, shard, in_dt)
    nc = _get_program(shard, in_dt, cache, out_dt=CONFIG.get("out_dt", "float32"),
                      dma=CONFIG.get("dma", "sync"),
                      tpb=int(CONFIG.get("tpb", 1)) if shard == "channel" else 1)

    old_m = nc.m
    nc.m = get_hw_module(nc.m)
    try:
        res = bass_utils.run_bass_kernel_spmd(nc, in_maps, core_ids=list(range(N_CORES)))
    finally:
        nc.m = old_m

    return _assemble_out(
        shard,
        [res.results[k]["y"] for k in range(N_CORES)],
        [res.results[k]["kl"] for k in range(N_CORES)],
    )


# revision 28
# speedup vs baseline: 1.4618x; 1.0879x over previous
"""BayesianBatchNorm on 8 TRN2 NeuronCores.

Two sharding strategies:

"channel" (default): each core owns 32 channels across the FULL batch, so
  the batch statistics complete locally — no cross-core communication.
  Host lays the core's data out as 16 tiles of [128, 3136] where
  partition p = (batch_quarter, channel) = (p//32, p%32); a single
  TensorE matmul against a 0/1 selection matrix reduces the per-quarter
  partial sums across partitions AND broadcasts the result back to all
  128 partitions. The KL term for the core's 32 channels ships out and
  the host sums the 8x32 terms into the div scalar.

"batch": data-parallel over batch; per-channel (sum, sumsq) partials are
  AllReduced across cores (2KB payload). Kept for comparison — the
  collective costs 25-55us wall on this environment.

Pass 2 normalizes y = x*scale + shift per channel and streams out.
"""

import sys

sys.path.insert(0, "/opt/trn_rl_repo")

import numpy as np

from concourse import bass, mybir
import concourse.bacc as bacc
import concourse.tile as tile
from concourse import bass_utils
from concourse.bass_interp import get_hw_module

N_CORES = 8
N, C, H, W = 64, 256, 56, 56
FREE = H * W                     # 3136
NT = 16                          # tiles of [128, FREE] per core
CPC = C // N_CORES               # 32 channels per core (channel mode)
NQ = 128 // CPC                  # 4 batch quarters on partitions (channel mode)
NB = N // N_CORES                # 8 batches per core (batch mode)
N_GLOBAL = N * FREE              # 200704 elements per channel
MOMENTUM = 0.1
EPS = 1e-5
JITTER = 1e-5

# consts tensor column layout
(C_A1, C_A2, C_B1, C_B2, C_LRM, C_VS, C_IVS, C_GAM, C_BET, C_Q, C_R) = range(11)
NCONST = 11

# shard: "channel" (no collective) or "batch" (AllReduce).
# in_dt: dtype x is stored in DRAM as (host casts).
# cache: keep x tiles resident in SBUF between the two passes (skips the
# second HBM read of x).
CONFIG = dict(shard="channel", in_dt="float16", cache=True, out_dt="float32", tpb=2)

_ALU = mybir.AluOpType
_AF = mybir.ActivationFunctionType
_F32 = mybir.dt.float32


def _build(shard: str, in_dt: str, cache: bool, variant: str = "full",
           out_dt: str = "float32", dma: str = "sync_gpsimd", tpb: int = 1):
    ng = 2 if shard == "batch" else 1
    assert tpb == 1 or (shard == "channel" and NT % tpb == 0)
    nt = NT // tpb
    fr = FREE * tpb
    nc = bacc.Bacc("TRN2", debug=False, enable_asserts=False, num_devices=N_CORES)
    xdt = {"float16": mybir.dt.float16, "bfloat16": mybir.dt.bfloat16}.get(in_dt, _F32)
    ydt = mybir.dt.float16 if out_dt == "float16" else _F32

    x = nc.dram_tensor("x", [nt, 128, fr], xdt, kind="ExternalInput").ap()
    cvec = nc.dram_tensor("cvec", [128, ng, NCONST], _F32, kind="ExternalInput").ap()
    y = nc.dram_tensor("y", [nt, 128, fr], ydt, kind="ExternalOutput").ap()
    klp = CPC if shard == "channel" else 128
    kl = nc.dram_tensor("kl", [klp, ng], _F32, kind="ExternalOutput").ap()
    if shard == "channel":
        sel = nc.dram_tensor("sel", [128, 128], _F32, kind="ExternalInput").ap()

    vec = nc.vector
    act = nc.scalar
    odd_eng = {"sync": nc.sync, "sync_scalar": nc.scalar, "sync_gpsimd": nc.gpsimd}[dma]

    with tile.TileContext(nc) as tc:
        with (
            tc.tile_pool(name="xin", bufs=nt if cache else 6) as xpool,
            tc.tile_pool(name="yout", bufs=4 if out_dt == "float16" or tpb == 1 else 2) as ypool,
            tc.tile_pool(name="small", bufs=1) as sp,
            tc.tile_pool(name="psum", bufs=1, space="PSUM") as pp,
            tc.tile_pool(name="dram", bufs=1, space="DRAM") as dp,
        ):
            cv = sp.tile([128, ng, NCONST], _F32)
            if shard == "channel":
                sel_t = sp.tile([128, 128], _F32)

            def cc(k):  # [128, ng] column view of the consts
                return cv[:, :, k]

            # ---- pass 1: load tiles, per-tile (sum, sumsq) partials ----
            npart = nt // ng
            sum_part = [
                sp.tile([128, npart], _F32, tag=f"sp{g}", name=f"sum_part{g}")
                for g in range(ng)
            ]
            sq_part = [
                sp.tile([128, npart], _F32, tag=f"qp{g}", name=f"sq_part{g}")
                for g in range(ng)
            ]
            xtiles = []
            for t in range(nt):
                g, nn = t % ng, t // ng
                xt = xpool.tile([128, fr], xdt, tag="xt")
                (nc.sync if t % 2 == 0 else odd_eng).dma_start(xt[:], x[t])
                vec.reduce_sum(
                    sum_part[g][:, nn : nn + 1], xt[:], axis=mybir.AxisListType.X
                )
                scr = sp.tile(
                    [128, fr], mybir.dt.float16, tag="scr", bufs=2, name="scr"
                )
                act.activation(
                    scr[:],
                    xt[:],
                    _AF.Square,
                    bias=0.0,
                    scale=1.0,
                    accum_out=sq_part[g][:, nn : nn + 1],
                )
                xtiles.append(xt)

            # consts/sel load late: needed only from the combine step on,
            # and issuing them first would delay the x loads
            nc.gpsimd.dma_start(cv[:], cvec)
            if shard == "channel":
                nc.gpsimd.dma_start(sel_t[:], sel)

            # ---- combine partials into global per-channel (sum, sumsq) ----
            cc_out = sp.tile([128, 2 * ng], _F32)
            if shard == "channel":
                packed = sp.tile([128, 2], _F32)
                vec.reduce_sum(packed[:, 0:1], sum_part[0][:], axis=mybir.AxisListType.X)
                vec.reduce_sum(packed[:, 1:2], sq_part[0][:], axis=mybir.AxisListType.X)
                # one matmul: sel[k,m]=1 iff k%32==m%32 reduces the 4 batch
                # quarters AND broadcasts back to all 128 partitions
                ps = pp.tile([128, 2], _F32)
                nc.tensor.matmul(ps[:], sel_t[:], packed[:], start=True, stop=True)
                vec.tensor_copy(cc_out[:], ps[:])
            else:
                cc_in = sp.tile([128, 2 * ng], _F32)
                for g in range(ng):
                    vec.reduce_sum(
                        cc_in[:, g : g + 1], sum_part[g][:], axis=mybir.AxisListType.X
                    )
                    vec.reduce_sum(
                        cc_in[:, ng + g : ng + g + 1],
                        sq_part[g][:],
                        axis=mybir.AxisListType.X,
                    )
                bounce_in = dp.tile([128, 2 * ng], _F32)
                bounce_out = dp.tile([128, 2 * ng], _F32)
                nc.gpsimd.dma_start(bounce_in[:], cc_in[:])
                if "nocc" in variant:
                    nc.gpsimd.dma_start(bounce_out[:], bounce_in[:])
                else:
                    nc.gpsimd.collective_compute(
                        "AllReduce",
                        _ALU.add,
                        replica_groups=[list(range(N_CORES))],
                        ins=[bounce_in.opt()],
                        outs=[bounce_out.opt()],
                    )
                nc.gpsimd.dma_start(cc_out[:], bounce_out[:])

            # ---- finalize: all [128, ng] elementwise ----
            sums, sqs = cc_out[:, 0:ng], cc_out[:, ng : 2 * ng]
            mean = sp.tile([128, ng], _F32)
            e2 = sp.tile([128, ng], _F32)
            bvar = sp.tile([128, ng], _F32)
            rmt = sp.tile([128, ng], _F32)
            rvt = sp.tile([128, ng], _F32)
            d = sp.tile([128, ng], _F32)
            d2 = sp.tile([128, ng], _F32)
            rmean = sp.tile([128, ng], _F32)
            rvar = sp.tile([128, ng], _F32)
            tmp = sp.tile([128, ng], _F32)
            std = sp.tile([128, ng], _F32)
            scal = sp.tile([128, ng], _F32)
            shif = sp.tile([128, ng], _F32)
            vt = sp.tile([128, ng], _F32)
            ivt = sp.tile([128, ng], _F32)
            r1 = sp.tile([128, ng], _F32)
            r2 = sp.tile([128, ng], _F32)
            siv = sp.tile([128, ng], _F32)
            klt = sp.tile([128, ng], _F32)

            # critical path to (scal, shif) first; KL afterwards
            vec.tensor_scalar_mul(mean[:], sums, 1.0 / N_GLOBAL)
            vec.tensor_scalar_mul(e2[:], sqs, 1.0 / N_GLOBAL)
            vec.tensor_mul(bvar[:], mean[:], mean[:])
            vec.tensor_sub(bvar[:], e2[:], bvar[:])
            # rm_t = 0.9*nrm + 0.1*mean ; rv_t = 0.9*nrv + (0.1*n/(n-1))*bvar
            vec.tensor_scalar_mul(rmt[:], mean[:], MOMENTUM)
            vec.tensor_add(rmt[:], rmt[:], cc(C_A1))
            vec.tensor_scalar_mul(rvt[:], bvar[:], MOMENTUM * N_GLOBAL / (N_GLOBAL - 1))
            vec.tensor_add(rvt[:], rvt[:], cc(C_A2))
            vec.tensor_sub(d[:], cc(C_LRM), rmt[:])
            vec.tensor_mul(d2[:], d[:], d[:])
            # running_mean = B1 + Q*rm_t ; running_var = B2 + Q*rv_t + R*d2
            vec.tensor_mul(rmean[:], cc(C_Q), rmt[:])
            vec.tensor_add(rmean[:], rmean[:], cc(C_B1))
            vec.tensor_mul(rvar[:], cc(C_Q), rvt[:])
            vec.tensor_add(rvar[:], rvar[:], cc(C_B2))
            vec.tensor_mul(tmp[:], cc(C_R), d2[:])
            vec.tensor_add(rvar[:], rvar[:], tmp[:])
            # scale = gamma / sqrt(running_var + eps); shift = beta - rmean*scale
            vec.tensor_scalar_add(rvar[:], rvar[:], EPS)
            act.activation(std[:], rvar[:], _AF.Sqrt, bias=0.0, scale=1.0)
            vec.reciprocal(std[:], std[:])
            vec.tensor_mul(scal[:], cc(C_GAM), std[:])
            vec.tensor_mul(shif[:], rmean[:], scal[:])
            vec.tensor_sub(shif[:], cc(C_BET), shif[:])

            # ---- pass 2: y = x*scale + shift ----
            for t in range(0 if "pass1" in variant else nt):
                g = t % ng
                if cache:
                    xt = xtiles[t]
                else:
                    xt = xpool.tile([128, fr], xdt, tag="xt")
                    (nc.sync if t % 2 == 0 else odd_eng).dma_start(xt[:], x[t])
                s_ap, b_ap = scal[:, g : g + 1], shif[:, g : g + 1]
                if xdt == ydt and not cache:
                    if t % 2 == 0:
                        act.activation(
                            xt[:], xt[:], _AF.Identity, bias=b_ap, scale=s_ap
                        )
                    else:
                        vec.tensor_scalar(
                            xt[:], xt[:], s_ap, b_ap, _ALU.mult, _ALU.add
                        )
                    (nc.sync if t % 2 == 0 else odd_eng).dma_start(y[t], xt[:])
                else:
                    yt = ypool.tile([128, fr], ydt, tag="yt")
                    if t % 2 == 0:
                        act.activation(
                            yt[:], xt[:], _AF.Identity, bias=b_ap, scale=s_ap
                        )
                    else:
                        vec.tensor_scalar(
                            yt[:], xt[:], s_ap, b_ap, _ALU.mult, _ALU.add
                        )
                    (nc.sync if t % 2 == 0 else odd_eng).dma_start(y[t], yt[:])

            # ---- KL terms (off the critical path) ----
            # kl_c = 0.25*(vs/vt + vt/vs + d2*(1/vs + 1/vt) - 2)
            vec.tensor_scalar_add(vt[:], rvt[:], JITTER)
            vec.reciprocal(ivt[:], vt[:])
            vec.tensor_mul(r1[:], cc(C_VS), ivt[:])
            vec.tensor_mul(r2[:], vt[:], cc(C_IVS))
            vec.tensor_add(siv[:], cc(C_IVS), ivt[:])
            vec.tensor_mul(siv[:], d2[:], siv[:])
            vec.tensor_add(klt[:], r1[:], r2[:])
            vec.tensor_add(klt[:], klt[:], siv[:])
            vec.tensor_scalar(klt[:], klt[:], 0.25, -0.5, _ALU.mult, _ALU.add)
            # gpsimd (SWDGE), NOT sync: a sync-queue store here would
            # head-of-line-block pass-2 DMAs behind the finalize chain
            nc.gpsimd.dma_start(kl, klt[:klp, :])

    nc.compile()
    return nc


_PROGRAM_CACHE = {}


def _get_program(shard: str, in_dt: str, cache: bool, variant: str = "full",
                 out_dt: str = "float32", dma: str = "sync_gpsimd", tpb: int = 1):
    key = (shard, in_dt, cache, variant, out_dt, dma, tpb)
    if key not in _PROGRAM_CACHE:
        _PROGRAM_CACHE[key] = _build(shard, in_dt, cache, variant, out_dt, dma, tpb)
    return _PROGRAM_CACHE[key]


def _const_cols(inputs, p):
    lrm = np.asarray(inputs["layer_running_mean"], np.float32)
    lrv = np.asarray(inputs["layer_running_var"], np.float32)
    gam = np.asarray(inputs["layer_weight"], np.float32)
    bet = np.asarray(inputs["layer_bias"], np.float32)
    nrm = np.asarray(inputs["norm_running_mean"], np.float32)
    nrv = np.asarray(inputs["norm_running_var"], np.float32)
    vs = lrv + np.float32(JITTER)
    cols = np.zeros((C, NCONST), np.float32)
    cols[:, C_A1] = (1.0 - MOMENTUM) * nrm
    cols[:, C_A2] = (1.0 - MOMENTUM) * nrv
    cols[:, C_B1] = p * lrm
    cols[:, C_B2] = p * lrv
    cols[:, C_LRM] = lrm
    cols[:, C_VS] = vs
    cols[:, C_IVS] = 1.0 / vs
    cols[:, C_GAM] = gam
    cols[:, C_BET] = bet
    cols[:, C_Q] = 1.0 - p
    cols[:, C_R] = p * (1.0 - p)
    return cols


def _prepare_in_maps(inputs, shard, in_dt):
    x = np.asarray(inputs["input"], np.float32)
    assert x.shape == (N, C, H, W), x.shape
    p = float(np.asarray(inputs["prior"], np.float32)[0])
    cols = _const_cols(inputs, p)
    if in_dt == "bfloat16":
        import ml_dtypes
        xdt = ml_dtypes.bfloat16
    else:
        xdt = np.float16 if in_dt == "float16" else np.float32
    xr = x.reshape(N, C, FREE)
    in_maps = []
    if shard == "channel":
        tpb = int(CONFIG.get("tpb", 1))
        nt = NT // tpb
        ii = np.arange(128)
        sel = (ii[:, None] % CPC == ii[None, :] % CPC).astype(np.float32)
        for k in range(N_CORES):
            ck = slice(k * CPC, (k + 1) * CPC)
            # [64, 32, F] -> tiles [nt, (quarter, channel)=128, tpb*F]
            xs = (
                xr[:, ck, :]
                .reshape(NQ, nt, tpb, CPC, FREE)
                .transpose(1, 0, 3, 2, 4)
                .reshape(nt, 128, tpb * FREE)
            )
            in_maps.append({
                "x": np.ascontiguousarray(xs, dtype=xdt),
                "cvec": np.ascontiguousarray(
                    np.tile(cols[ck], (NQ, 1))[:, None, :]
                ),
                "sel": sel,
            })
    else:
        consts = np.ascontiguousarray(
            cols.reshape(2, 128, NCONST).transpose(1, 0, 2)
        )
        for k in range(N_CORES):
            shard_x = xr[k * NB : (k + 1) * NB].reshape(NT, 128, FREE)
            in_maps.append({
                "x": np.ascontiguousarray(shard_x, dtype=xdt),
                "cvec": consts,
            })
    return in_maps


def _assemble_out(shard, per_core_y, per_core_kl):
    out = np.empty((N, C, FREE), np.float32)
    if shard == "channel":
        tpb = int(CONFIG.get("tpb", 1))
        nt = NT // tpb
        for k in range(N_CORES):
            yk = np.asarray(per_core_y[k]).reshape(nt, NQ, CPC, tpb, FREE)
            out[:, k * CPC : (k + 1) * CPC, :] = (
                yk.transpose(1, 0, 3, 2, 4).reshape(N, CPC, FREE)
            )
        div = np.float32(
            sum(np.asarray(klk, np.float64).sum() for klk in per_core_kl)
        )
    else:
        for k in range(N_CORES):
            out[k * NB : (k + 1) * NB] = np.asarray(per_core_y[k]).reshape(
                NB, C, FREE
            )
        div = np.float32(np.asarray(per_core_kl[0], np.float64).sum())
    return out.reshape(N, C, H, W), div


def kernel(**inputs):
    shard, in_dt, cache = CONFIG["shard"], CONFIG["in_dt"], CONFIG["cache"]
    in_maps = _prepare_in_maps(inputs, shard, in_dt)
    nc = _get_program(shard, in_dt, cache, out_dt=CONFIG.get("out_dt", "float32"),
                      dma=CONFIG.get("dma", "sync"),
                      tpb=int(CONFIG.get("tpb", 1)) if shard == "channel" else 1)

    old_m = nc.m
    nc.m = get_hw_module(nc.m)
    try:
        res = bass_utils.run_bass_kernel_spmd(nc, in_maps, core_ids=list(range(N_CORES)))
    finally:
        nc.m = old_m

    return _assemble_out(
        shard,
        [res.results[k]["y"] for k in range(N_CORES)],
        [res.results[k]["kl"] for k in range(N_CORES)],
    )
